# revision 10
# baseline (speedup 1.0000x reference)
"""Trainium2 Bass kernel: multi-head self-attention block (B=16, N=1024, C=768, H=12).

Data-parallel over batch: 8 NeuronCores x 2 batches each, no collectives.

Dataflow (per core, all-transposed activations; no on-chip transposes):
  host: xT = x_shard^T                                  [C, T]
  qkT  = W_qkv[:, :2C]^T-tiles @ xT                     [2C, T]   (q^T | k^T)
  v'   = xT-tiles^T @ W_qkv[:, 2C:]  (+ ones col/head)  [T, H*(HD+1)]
  S^T  = k^T-slices^T @ q^T   (per head, K=64)          [Nk, Nq]
  E    = exp(SCALE * S^T)     (ScalarE, PSUM->SBUF)
  U'   = v'^T @ E  (accum over k; row HD = softmax Z)   [HD+1, Nq]
  aoT  = U'[:HD] * (1/Z broadcast)                      [C, T]
  y    = aoT-tiles^T @ W_proj + b                       [T, C]

Schedule strategy: the S->EXP chain is the pacing conveyor (Scalar EXP is
192 x ~1.1us serial).  E tiles are buffered ~2 head-pairs deep so the U
matmuls decouple from the conveyor and run as fill work, in two 512-wide
passes per (hp,b) so only 2 U accumulator banks are live (instead of 4).
The freed banks form a dedicated filler PSUM pool so qkv-proj / v-phase /
out-proj matmul groups are never slot-blocked and can fill conveyor slack.
Batch-outer loop ordering lets proj(b0) overlap b1's attention.
"""

import sys

for _p in ("/opt/trn_rl_repo", "/opt/pypackages"):
    if _p not in sys.path:
        sys.path.append(_p)

import numpy as np

B, N, C, H = 16, 1024, 768, 12
HD = C // H            # 64
SCALE = HD ** -0.5
NCORES = 8
BL = B // NCORES       # 2 batches per core
T = BL * N             # 2048 tokens per core

COMPUTE = "bf16"       # "bf16" | "f32" | "f32r"


def build_attention_nc(compute=COMPUTE, bl=BL, n=N, c=C, h=H):
    import concourse.bass as bass
    import concourse.tile as tile
    from concourse import bacc, mybir
    from contextlib import ExitStack

    hd = c // h
    t = bl * n
    scale = hd ** -0.5
    assert c % 128 == 0 and n % 512 == 0 and h % 2 == 0 and hd == 64
    CCH = c // 128      # contraction chunks over channels
    NHP = h // 2        # head pairs
    NQ = n // 512       # 512-wide q tiles per sequence
    NKT = n // 128      # 128-wide k tiles per sequence
    NTT = n // 128      # 128-wide token tiles per sequence
    VW = hd + 1         # v' width per head (ones col at hd)
    PH = c // 2         # proj/v free-dim half (768/2=384), <= 512 & <= 1 PSUM bank
    assert PH <= 512

    FP32 = mybir.dt.float32
    SD = mybir.dt.bfloat16 if compute == "bf16" else FP32  # storage dtype

    def mm(ap):
        # matmul-operand view; f32r = fast single-pass fp32 path on TRN2 PE
        return ap.bitcast(mybir.dt.float32r) if compute == "f32r" else ap

    nc = bacc.Bacc("TRN2", target_bir_lowering=False, debug=False,
                   num_devices=NCORES)

    # inputs arrive pre-cast to the storage dtype (host-side cast)
    xT_d = nc.dram_tensor("xT", [c, t], SD, kind="ExternalInput").ap()
    wqkv_d = nc.dram_tensor("w_qkv", [c, 3 * c], SD, kind="ExternalInput").ap()
    wproj_d = nc.dram_tensor("w_proj", [c, c], SD, kind="ExternalInput").ap()
    bias_d = nc.dram_tensor("bias", [128, c], FP32, kind="ExternalInput").ap()
    out_d = nc.dram_tensor("out", [t, c], FP32, kind="ExternalOutput").ap()

    Exp = mybir.ActivationFunctionType.Exp

    with tile.TileContext(nc) as tc, ExitStack() as ctx:
        consts = ctx.enter_context(tc.tile_pool(name="consts", bufs=1))
        xp = ctx.enter_context(tc.tile_pool(name="xp", bufs=1))
        qkp = ctx.enter_context(tc.tile_pool(name="qkp", bufs=2))
        vp = ctx.enter_context(tc.tile_pool(name="vp", bufs=2))
        ep = ctx.enter_context(tc.tile_pool(name="ep", bufs=18))
        aop = ctx.enter_context(tc.tile_pool(name="aop", bufs=2))
        smp = ctx.enter_context(tc.tile_pool(name="smp", bufs=2))
        yp = ctx.enter_context(tc.tile_pool(name="yp", bufs=4))
        ps_s = ctx.enter_context(tc.tile_pool(name="ps_s", bufs=3, space="PSUM"))
        ps_u = ctx.enter_context(tc.tile_pool(name="ps_u", bufs=2, space="PSUM"))

        # --- input DMAs spread across four issue rings so the v-phase can
        # start ~9.5us in: b0-x xh0 halves split sync/vector, xh1 on scalar,
        # wv halves on gpsimd (half0 first); then wqk (sync, needed by
        # qkproj(hp0) ~25us in), batch-1 x (gpsimd), wproj / bias last. ---
        NXH = n // 512
        wqk_sb = []
        wv_sb = []
        xT_all = [[[None] * NXH for _ in range(bl)] for _ in range(CCH)]
        for cc in range(CCH):
            eng = nc.sync if cc < CCH // 2 else nc.scalar
            xt = xp.tile([128, 512], SD, tag=f"x{cc}_0_0", name=f"x_b0c{cc}h0")
            eng.dma_start(out=xt, in_=xT_d[cc * 128:(cc + 1) * 128, 0:512])
            xT_all[cc][0][0] = xt
            wv = consts.tile([128, c], SD, tag=f"wv{cc}")
            nc.gpsimd.dma_start(out=wv[:, 0:PH],
                                in_=wqkv_d[cc * 128:(cc + 1) * 128,
                                           2 * c:2 * c + PH])
            wv_sb.append(wv)
        for cc in range(CCH):
            for xh in range(1, NXH):
                xt = xp.tile([128, 512], SD, tag=f"x{cc}_0_{xh}",
                             name=f"x_b0c{cc}h{xh}")
                nc.scalar.dma_start(
                    out=xt, in_=xT_d[cc * 128:(cc + 1) * 128,
                                     xh * 512:(xh + 1) * 512])
                xT_all[cc][0][xh] = xt
            nc.gpsimd.dma_start(out=wv_sb[cc][:, PH:c],
                                in_=wqkv_d[cc * 128:(cc + 1) * 128,
                                           2 * c + PH:3 * c])
        for cc in range(CCH):
            w1 = consts.tile([128, 2 * c], SD, tag=f"wqkv{cc}")
            nc.sync.dma_start(out=w1, in_=wqkv_d[cc * 128:(cc + 1) * 128,
                                                 0:2 * c])
            wqk_sb.append(w1)
        for cc in range(CCH):
            for xh in range(NXH):
                for b in range(1, bl):
                    xt = xp.tile([128, 512], SD, tag=f"x{cc}_{b}_{xh}",
                                 name=f"x_b{b}c{cc}h{xh}")
                    nc.gpsimd.dma_start(
                        out=xt, in_=xT_d[cc * 128:(cc + 1) * 128,
                                         b * n + xh * 512:b * n + (xh + 1) * 512])
                    xT_all[cc][b][xh] = xt
        wproj_sb = []
        for cc in range(CCH):
            w2 = consts.tile([128, c], SD, tag=f"wproj{cc}")
            nc.sync.dma_start(out=w2, in_=wproj_d[cc * 128:(cc + 1) * 128, :])
            wproj_sb.append(w2)
        bias_sb = consts.tile([128, c], FP32, tag="bias")
        nc.gpsimd.dma_start(out=bias_sb, in_=bias_d)

        # --- v' tiles, both batches up front (b1's fills b0 conveyor slack):
        # [128 tok, h*VW], ones col per head at hd ---
        v_all = [[None] * NTT for _ in range(bl)]
        for b in range(bl):
            for tt in range(NTT):
                vt = vp.tile([128, h * VW], SD, tag=f"v{tt}", name=f"v_b{b}t{tt}")
                ones_view = vt[:, :].rearrange("p (hh w) -> p hh w", hh=h)[:, :, hd:hd + 1]
                nc.gpsimd.memset(ones_view, 1.0)
                for half in range(2):
                    ps = ps_s.tile([128, 1024], FP32, tag="s",
                                   name=f"vps_b{b}t{tt}f{half}")
                    for cc in range(CCH):
                        xh, tl = tt // 4, tt % 4
                        nc.tensor.matmul(
                            ps[:, 0:PH],
                            lhsT=mm(xT_all[cc][b][xh][:, tl * 128:(tl + 1) * 128]),
                            rhs=mm(wv_sb[cc][:, half * PH:(half + 1) * PH]),
                            start=(cc == 0), stop=(cc == CCH - 1))
                    # strided copy into per-head 64-wide slices (skip ones col)
                    nheads = PH // hd
                    dst = vt[:, half * nheads * VW:(half + 1) * nheads * VW].rearrange(
                        "p (hh w) -> p hh w", hh=nheads)[:, :, 0:hd]
                    srcv = ps[:, 0:PH].rearrange("p (hh w) -> p hh w", hh=nheads)
                    with tc.high_priority(offset=300):
                        nc.vector.tensor_copy(dst, srcv)
                v_all[b][tt] = vt

        # --- batch-outer: per (b, hp): qkproj then attention; proj(b) right
        # after so it fills the next batch's conveyor slack ---
        aoT_all = [[] for _ in range(bl)]
        for b in range(bl):
            for hp in range(NHP):
                # q^T pair tile (2 heads stacked) and k^T pair tile
                qt = qkp.tile([128, n], SD, tag="qt", name=f"qt{b}_{hp}")
                kt_ = qkp.tile([128, n], SD, tag="kt", name=f"kt{b}_{hp}")
                for dst, fbase in ((qt, hp * 128), (kt_, c + hp * 128)):
                    for qn in range(NQ):
                        ps = ps_s.tile([128, 1024], FP32, tag="s",
                                       name=f"qkps{b}_{hp}_{qn}")
                        for cc in range(CCH):
                            nc.tensor.matmul(
                                ps[:, 0:512],
                                lhsT=mm(wqk_sb[cc][:, fbase:fbase + 128]),
                                rhs=mm(xT_all[cc][b][qn]),
                                start=(cc == 0), stop=(cc == CCH - 1))
                        with tc.high_priority(offset=300):
                            nc.vector.tensor_copy(dst[:, qn * 512:(qn + 1) * 512], ps[:, 0:512])

                # --- S -> EXP conveyor: S matmuls at high priority so the
                # Scalar engine never starves; E buffered deep in ep ---
                ets = [[None, None] for _ in range(NKT)]
                for kt in range(NKT):
                    sps_l = []
                    for head in range(2):
                        p0 = head * 64
                        sps = ps_s.tile([128, n], FP32, tag="s",
                                        name=f"s_b{b}hp{hp}k{kt}h{head}")
                        for qn in range(NQ):
                            with tc.high_priority(offset=600):
                                nc.tensor.matmul(
                                    sps[:, qn * 512:(qn + 1) * 512],
                                    lhsT=mm(kt_[p0:p0 + 64, kt * 128:(kt + 1) * 128]),
                                    rhs=mm(qt[p0:p0 + 64, qn * 512:(qn + 1) * 512]),
                                    start=True, stop=True)
                        sps_l.append(sps)
                    for head in range(2):
                        et = ep.tile([128, n], SD, tag="e",
                                     name=f"e_b{b}hp{hp}k{kt}h{head}")
                        nc.scalar.activation(et, sps_l[head], Exp, scale=scale)
                        ets[kt][head] = et

                # --- U: two 512-wide passes over the buffered E tiles; only
                # one accumulator bank per head is live at a time ---
                usb = [smp.tile([VW, n], FP32, tag=f"usb{head}",
                                name=f"usb_b{b}hp{hp}h{head}")
                       for head in range(2)]
                for qn in range(NQ):
                    u_ps = [ps_u.tile([VW, 512], FP32, tag="u",
                                      name=f"u_b{b}hp{hp}q{qn}h{hh}")
                            for hh in range(2)]
                    for kt in range(NKT):
                        for head in range(2):
                            hh = 2 * hp + head
                            nc.tensor.matmul(
                                u_ps[head],
                                lhsT=mm(v_all[b][kt][:, hh * VW:hh * VW + VW]),
                                rhs=mm(ets[kt][head][:, qn * 512:(qn + 1) * 512]),
                                start=(kt == 0), stop=(kt == NKT - 1))
                    for head in range(2):
                        # these copies gate the U-accumulator bank release:
                        # jump the DVE queue
                        with tc.high_priority(offset=300):
                            nc.vector.tensor_copy(
                                usb[head][:, qn * 512:(qn + 1) * 512],
                                u_ps[head])

                # normalize: aoT[hp] rows 0:64 = head A, 64:128 = head B.
                ao = aop.tile([128, n], SD, tag=f"ao{hp}", name=f"ao_b{b}hp{hp}")
                for head in (1, 0):
                    # Z row -> partition 0 (DMA), broadcast to 64 partitions
                    # (gpsimd), then reciprocal on the full-width tile (the
                    # custom DVE op mis-executes on 1-partition slices at
                    # base partition != 0).
                    z1 = smp.tile([1, n], FP32, tag=f"z1{head}", bufs=1,
                                  name=f"z1_b{b}hp{hp}h{head}")
                    nc.gpsimd.dma_start(out=z1, in_=usb[head][hd:hd + 1, :])
                    rb = smp.tile([64, n], FP32, tag=f"rb{head}", bufs=1,
                                  name=f"rb_b{b}hp{hp}h{head}")
                    nc.gpsimd.partition_broadcast(rb, z1)
                    nc.vector.reciprocal_approx_fast(rb, rb)
                    if head == 0:
                        nc.vector.tensor_mul(ao[0:64, :], usb[0][0:hd, :], rb)
                    else:
                        sc = smp.tile([64, n], SD, tag="sc", bufs=1,
                                      name=f"sc_b{b}hp{hp}")
                        nc.vector.tensor_mul(sc, usb[1][0:hd, :], rb)
                        nc.gpsimd.dma_start(out=ao[64:128, :], in_=sc)
                aoT_all[b].append(ao)

            # --- output projection + bias for this batch.  Split: chunks
            # hp0-4 accumulate early (they only need the first 5 ao tiles, so
            # they fill the last head-pair's conveyor slack); the hp5 chunk is
            # a single matmul + add once the final normalize lands, so the
            # batch tail is short. ---
            partials = []
            for tt in range(NTT):
                for half in range(2):
                    ps = ps_s.tile([128, 1024], FP32, tag="s",
                                   name=f"ypp_b{b}t{tt}f{half}")
                    for cc in range(CCH - 1):
                        nc.tensor.matmul(
                            ps[:, 0:PH],
                            lhsT=mm(aoT_all[b][cc][:, tt * 128:(tt + 1) * 128]),
                            rhs=mm(wproj_sb[cc][:, half * PH:(half + 1) * PH]),
                            start=(cc == 0), stop=(cc == CCH - 2))
                    pp = yp.tile([128, PH], SD, tag=f"pp{tt}_{half}", bufs=1,
                                 name=f"pp_b{b}t{tt}f{half}")
                    with tc.high_priority(offset=300):
                        nc.vector.tensor_add(pp, ps[:, 0:PH],
                                             bias_sb[:, half * PH:(half + 1) * PH])
                    partials.append(pp)
            cl = CCH - 1
            for tt in range(NTT):
                for half in range(2):
                    ps = ps_s.tile([128, 1024], FP32, tag="s",
                                   name=f"yps_b{b}t{tt}f{half}")
                    nc.tensor.matmul(
                        ps[:, 0:PH],
                        lhsT=mm(aoT_all[b][cl][:, tt * 128:(tt + 1) * 128]),
                        rhs=mm(wproj_sb[cl][:, half * PH:(half + 1) * PH]),
                        start=True, stop=True)
                    yt = yp.tile([128, PH], FP32, tag="y", name=f"y_b{b}t{tt}f{half}")
                    with tc.high_priority(offset=300):
                        nc.vector.tensor_add(yt, ps[:, 0:PH],
                                             partials[tt * 2 + half])
                    nc.sync.dma_start(
                        out=out_d[b * n + tt * 128:b * n + (tt + 1) * 128,
                                  half * PH:(half + 1) * PH],
                        in_=yt)

    nc.compile()
    return nc


_NC_CACHE = {}


def _get_nc(compute=COMPUTE):
    if compute not in _NC_CACHE:
        _NC_CACHE[compute] = build_attention_nc(compute)
    return _NC_CACHE[compute]


def make_in_maps(x, W_qkv, W_proj, b_proj, compute=None):
    compute = compute or COMPUTE
    if compute == "bf16":
        import ml_dtypes
        sd = ml_dtypes.bfloat16
    else:
        sd = np.float32
    x = np.asarray(x, dtype=np.float32)
    W_qkv = np.ascontiguousarray(np.asarray(W_qkv, dtype=np.float32)).astype(sd)
    W_proj = np.ascontiguousarray(np.asarray(W_proj, dtype=np.float32)).astype(sd)
    bias = np.ascontiguousarray(
        np.broadcast_to(np.asarray(b_proj, dtype=np.float32), (128, C)))
    in_maps = []
    for i in range(NCORES):
        shard = x[i * BL:(i + 1) * BL]                      # [BL, N, C]
        xT = np.ascontiguousarray(shard.transpose(2, 0, 1).reshape(C, T)).astype(sd)
        in_maps.append({"xT": xT, "w_qkv": W_qkv, "w_proj": W_proj,
                        "bias": bias})
    return in_maps


def kernel(x, W_qkv, W_proj, b_proj):
    from concourse.bass_utils import run_bass_kernel_spmd

    nc = _get_nc()
    in_maps = make_in_maps(x, W_qkv, W_proj, b_proj)
    res = run_bass_kernel_spmd(nc, in_maps, core_ids=list(range(NCORES)))
    outs = [res.results[i]["out"].reshape(BL, N, C) for i in range(NCORES)]
    return np.concatenate(outs, axis=0).astype(np.float32)


if __name__ == "__main__":
    nc = build_attention_nc()
    print("built ok")


# revision 11
# speedup vs baseline: 1.0472x; 1.0472x over previous
"""Trainium2 Bass kernel: multi-head self-attention block (B=16, N=1024, C=768, H=12).

Data-parallel over batch: 8 NeuronCores x 2 batches each, no collectives.

Dataflow (per core, all-transposed activations; no on-chip transposes):
  host: xT = x_shard^T                                  [C, T]
  qkT  = W_qkv[:, :2C]^T-tiles @ xT                     [2C, T]   (q^T | k^T)
  v'   = xT-tiles^T @ W_qkv[:, 2C:]  (+ ones col/head)  [T, H*(HD+1)]
  S^T  = k^T-slices^T @ q^T   (per head, K=64)          [Nk, Nq]
  E    = exp(SCALE * S^T)     (ScalarE, PSUM->SBUF)
  U'   = v'^T @ E  (accum over k; row HD = softmax Z)   [HD+1, Nq]
  aoT  = U'[:HD] * (1/Z broadcast)                      [C, T]
  y    = aoT-tiles^T @ W_proj + b                       [T, C]

Schedule strategy: the S->EXP chain is the pacing conveyor (Scalar EXP is
192 x ~1.1us serial).  E tiles are buffered ~2 head-pairs deep so the U
matmuls decouple from the conveyor and run as fill work, in two 512-wide
passes per (hp,b) so only 2 U accumulator banks are live (instead of 4).
The freed banks form a dedicated filler PSUM pool so qkv-proj / v-phase /
out-proj matmul groups are never slot-blocked and can fill conveyor slack.
Batch-outer loop ordering lets proj(b0) overlap b1's attention.
"""

import sys

for _p in ("/opt/trn_rl_repo", "/opt/pypackages"):
    if _p not in sys.path:
        sys.path.append(_p)

import numpy as np

B, N, C, H = 16, 1024, 768, 12
HD = C // H            # 64
SCALE = HD ** -0.5
NCORES = 8
BL = B // NCORES       # 2 batches per core
T = BL * N             # 2048 tokens per core

COMPUTE = "bf16"       # "bf16" | "f32" | "f32r"


def build_attention_nc(compute=COMPUTE, bl=BL, n=N, c=C, h=H):
    import concourse.bass as bass
    import concourse.tile as tile
    from concourse import bacc, mybir
    from contextlib import ExitStack

    hd = c // h
    t = bl * n
    scale = hd ** -0.5
    assert c % 128 == 0 and n % 512 == 0 and h % 2 == 0 and hd == 64
    CCH = c // 128      # contraction chunks over channels
    NHP = h // 2        # head pairs
    NQ = n // 512       # 512-wide q tiles per sequence
    NKT = n // 128      # 128-wide k tiles per sequence
    NTT = n // 128      # 128-wide token tiles per sequence
    VW = hd + 1         # v' width per head (ones col at hd)
    PH = c // 2         # proj/v free-dim half (768/2=384), <= 512 & <= 1 PSUM bank
    assert PH <= 512

    FP32 = mybir.dt.float32
    SD = mybir.dt.bfloat16 if compute == "bf16" else FP32  # storage dtype

    def mm(ap):
        # matmul-operand view; f32r = fast single-pass fp32 path on TRN2 PE
        return ap.bitcast(mybir.dt.float32r) if compute == "f32r" else ap

    nc = bacc.Bacc("TRN2", target_bir_lowering=False, debug=False,
                   num_devices=NCORES)

    # inputs arrive pre-cast to the storage dtype (host-side cast)
    xT_d = nc.dram_tensor("xT", [c, t], SD, kind="ExternalInput").ap()
    wqkv_d = nc.dram_tensor("w_qkv", [c, 3 * c], SD, kind="ExternalInput").ap()
    wproj_d = nc.dram_tensor("w_proj", [c, c], SD, kind="ExternalInput").ap()
    bias_d = nc.dram_tensor("bias", [128, c], FP32, kind="ExternalInput").ap()
    out_d = nc.dram_tensor("out", [t, c], FP32, kind="ExternalOutput").ap()

    Exp = mybir.ActivationFunctionType.Exp

    with tile.TileContext(nc) as tc, ExitStack() as ctx:
        consts = ctx.enter_context(tc.tile_pool(name="consts", bufs=1))
        xp = ctx.enter_context(tc.tile_pool(name="xp", bufs=1))
        qkp = ctx.enter_context(tc.tile_pool(name="qkp", bufs=2))
        vp = ctx.enter_context(tc.tile_pool(name="vp", bufs=2))
        ep = ctx.enter_context(tc.tile_pool(name="ep", bufs=18))
        aop = ctx.enter_context(tc.tile_pool(name="aop", bufs=2))
        smp = ctx.enter_context(tc.tile_pool(name="smp", bufs=2))
        yp = ctx.enter_context(tc.tile_pool(name="yp", bufs=4))
        ps_s = ctx.enter_context(tc.tile_pool(name="ps_s", bufs=2, space="PSUM"))
        ps_w = ctx.enter_context(tc.tile_pool(name="ps_w", bufs=2, space="PSUM"))
        ps_u = ctx.enter_context(tc.tile_pool(name="ps_u", bufs=2, space="PSUM"))

        # --- input DMAs spread across four issue rings so the v-phase can
        # start ~9.5us in: b0-x xh0 halves split sync/vector, xh1 on scalar,
        # wv halves on gpsimd (half0 first); then wqk (sync, needed by
        # qkproj(hp0) ~25us in), batch-1 x (gpsimd), wproj / bias last. ---
        NXH = n // 512
        wqk_sb = []
        wv_sb = []
        xT_all = [[[None] * NXH for _ in range(bl)] for _ in range(CCH)]
        for cc in range(CCH):
            eng = nc.sync if cc < CCH // 2 else nc.scalar
            xt = xp.tile([128, 512], SD, tag=f"x{cc}_0_0", name=f"x_b0c{cc}h0")
            eng.dma_start(out=xt, in_=xT_d[cc * 128:(cc + 1) * 128, 0:512])
            xT_all[cc][0][0] = xt
            wv = consts.tile([128, c], SD, tag=f"wv{cc}")
            nc.gpsimd.dma_start(out=wv[:, 0:PH],
                                in_=wqkv_d[cc * 128:(cc + 1) * 128,
                                           2 * c:2 * c + PH])
            wv_sb.append(wv)
        for cc in range(CCH):
            for xh in range(1, NXH):
                xt = xp.tile([128, 512], SD, tag=f"x{cc}_0_{xh}",
                             name=f"x_b0c{cc}h{xh}")
                nc.scalar.dma_start(
                    out=xt, in_=xT_d[cc * 128:(cc + 1) * 128,
                                     xh * 512:(xh + 1) * 512])
                xT_all[cc][0][xh] = xt
            nc.gpsimd.dma_start(out=wv_sb[cc][:, PH:c],
                                in_=wqkv_d[cc * 128:(cc + 1) * 128,
                                           2 * c + PH:3 * c])
        for cc in range(CCH):
            w1 = consts.tile([128, 2 * c], SD, tag=f"wqkv{cc}")
            nc.sync.dma_start(out=w1, in_=wqkv_d[cc * 128:(cc + 1) * 128,
                                                 0:2 * c])
            wqk_sb.append(w1)
        for cc in range(CCH):
            for xh in range(NXH):
                for b in range(1, bl):
                    xt = xp.tile([128, 512], SD, tag=f"x{cc}_{b}_{xh}",
                                 name=f"x_b{b}c{cc}h{xh}")
                    nc.gpsimd.dma_start(
                        out=xt, in_=xT_d[cc * 128:(cc + 1) * 128,
                                         b * n + xh * 512:b * n + (xh + 1) * 512])
                    xT_all[cc][b][xh] = xt
        wproj_sb = []
        for cc in range(CCH):
            w2 = consts.tile([128, c], SD, tag=f"wproj{cc}")
            nc.sync.dma_start(out=w2, in_=wproj_d[cc * 128:(cc + 1) * 128, :])
            wproj_sb.append(w2)
        bias_sb = consts.tile([128, c], FP32, tag="bias")
        nc.gpsimd.dma_start(out=bias_sb, in_=bias_d)

        # --- v' tiles, both batches up front (b1's fills b0 conveyor slack):
        # [128 tok, h*VW], ones col per head at hd ---
        v_all = [[None] * NTT for _ in range(bl)]
        for b in range(bl):
            for tt in range(NTT):
                vt = vp.tile([128, h * VW], SD, tag=f"v{tt}", name=f"v_b{b}t{tt}")
                ones_view = vt[:, :].rearrange("p (hh w) -> p hh w", hh=h)[:, :, hd:hd + 1]
                nc.gpsimd.memset(ones_view, 1.0)
                for half in range(2):
                    ps = ps_w.tile([128, 512], FP32, tag="w",
                                   name=f"vps_b{b}t{tt}f{half}")
                    for cc in range(CCH):
                        xh, tl = tt // 4, tt % 4
                        nc.tensor.matmul(
                            ps[:, 0:PH],
                            lhsT=mm(xT_all[cc][b][xh][:, tl * 128:(tl + 1) * 128]),
                            rhs=mm(wv_sb[cc][:, half * PH:(half + 1) * PH]),
                            start=(cc == 0), stop=(cc == CCH - 1))
                    # strided copy into per-head 64-wide slices (skip ones col)
                    nheads = PH // hd
                    dst = vt[:, half * nheads * VW:(half + 1) * nheads * VW].rearrange(
                        "p (hh w) -> p hh w", hh=nheads)[:, :, 0:hd]
                    srcv = ps[:, 0:PH].rearrange("p (hh w) -> p hh w", hh=nheads)
                    with tc.high_priority(offset=300):
                        nc.vector.tensor_copy(dst, srcv)
                v_all[b][tt] = vt

        # --- batch-outer: per (b, hp): qkproj then attention; proj(b) right
        # after so it fills the next batch's conveyor slack ---
        aoT_all = [[] for _ in range(bl)]
        for b in range(bl):
            for hp in range(NHP):
                # q^T pair tile (2 heads stacked) and k^T pair tile
                qt = qkp.tile([128, n], SD, tag="qt", name=f"qt{b}_{hp}")
                kt_ = qkp.tile([128, n], SD, tag="kt", name=f"kt{b}_{hp}")
                for dst, fbase in ((qt, hp * 128), (kt_, c + hp * 128)):
                    for qn in range(NQ):
                        ps = ps_w.tile([128, 512], FP32, tag="w",
                                       name=f"qkps{b}_{hp}_{qn}")
                        for cc in range(CCH):
                            nc.tensor.matmul(
                                ps,
                                lhsT=mm(wqk_sb[cc][:, fbase:fbase + 128]),
                                rhs=mm(xT_all[cc][b][qn]),
                                start=(cc == 0), stop=(cc == CCH - 1))
                        with tc.high_priority(offset=300):
                            nc.vector.tensor_copy(dst[:, qn * 512:(qn + 1) * 512], ps)

                # --- S -> EXP conveyor: S matmuls at high priority so the
                # Scalar engine never starves; E buffered deep in ep ---
                ets = [[None, None] for _ in range(NKT)]
                for kt in range(NKT):
                    sps_l = []
                    for head in range(2):
                        p0 = head * 64
                        sps = ps_s.tile([128, n], FP32, tag="s",
                                        name=f"s_b{b}hp{hp}k{kt}h{head}")
                        for qn in range(NQ):
                            with tc.high_priority(offset=600):
                                nc.tensor.matmul(
                                    sps[:, qn * 512:(qn + 1) * 512],
                                    lhsT=mm(kt_[p0:p0 + 64, kt * 128:(kt + 1) * 128]),
                                    rhs=mm(qt[p0:p0 + 64, qn * 512:(qn + 1) * 512]),
                                    start=True, stop=True)
                        sps_l.append(sps)
                    for head in range(2):
                        et = ep.tile([128, n], SD, tag="e",
                                     name=f"e_b{b}hp{hp}k{kt}h{head}")
                        nc.scalar.activation(et, sps_l[head], Exp, scale=scale)
                        ets[kt][head] = et

                # --- U: two 512-wide passes over the buffered E tiles; only
                # one accumulator bank per head is live at a time ---
                usb = [smp.tile([VW, n], FP32, tag=f"usb{head}",
                                name=f"usb_b{b}hp{hp}h{head}")
                       for head in range(2)]
                for qn in range(NQ):
                    u_ps = [ps_u.tile([VW, 512], FP32, tag="u",
                                      name=f"u_b{b}hp{hp}q{qn}h{hh}")
                            for hh in range(2)]
                    for kt in range(NKT):
                        for head in range(2):
                            hh = 2 * hp + head
                            nc.tensor.matmul(
                                u_ps[head],
                                lhsT=mm(v_all[b][kt][:, hh * VW:hh * VW + VW]),
                                rhs=mm(ets[kt][head][:, qn * 512:(qn + 1) * 512]),
                                start=(kt == 0), stop=(kt == NKT - 1))
                    for head in range(2):
                        # these copies gate the U-accumulator bank release:
                        # jump the DVE queue
                        with tc.high_priority(offset=300):
                            nc.vector.tensor_copy(
                                usb[head][:, qn * 512:(qn + 1) * 512],
                                u_ps[head])

                # normalize: aoT[hp] rows 0:64 = head A, 64:128 = head B.
                ao = aop.tile([128, n], SD, tag=f"ao{hp}", name=f"ao_b{b}hp{hp}")
                for head in (1, 0):
                    # Z row -> partition 0 (DMA), broadcast to 64 partitions
                    # (gpsimd), then reciprocal on the full-width tile (the
                    # custom DVE op mis-executes on 1-partition slices at
                    # base partition != 0).
                    z1 = smp.tile([1, n], FP32, tag=f"z1{head}", bufs=1,
                                  name=f"z1_b{b}hp{hp}h{head}")
                    nc.gpsimd.dma_start(out=z1, in_=usb[head][hd:hd + 1, :])
                    rb = smp.tile([64, n], FP32, tag=f"rb{head}", bufs=1,
                                  name=f"rb_b{b}hp{hp}h{head}")
                    nc.gpsimd.partition_broadcast(rb, z1)
                    nc.vector.reciprocal_approx_fast(rb, rb)
                    if head == 0:
                        nc.vector.tensor_mul(ao[0:64, :], usb[0][0:hd, :], rb)
                    else:
                        sc = smp.tile([64, n], SD, tag="sc", bufs=1,
                                      name=f"sc_b{b}hp{hp}")
                        nc.vector.tensor_mul(sc, usb[1][0:hd, :], rb)
                        nc.gpsimd.dma_start(out=ao[64:128, :], in_=sc)
                aoT_all[b].append(ao)

            # --- output projection + bias for this batch.  Split: chunks
            # hp0-4 accumulate early (they only need the first 5 ao tiles, so
            # they fill the last head-pair's conveyor slack); the hp5 chunk is
            # a single matmul + add once the final normalize lands, so the
            # batch tail is short. ---
            partials = []
            for tt in range(NTT):
                for half in range(2):
                    ps = ps_w.tile([128, 512], FP32, tag="w",
                                   name=f"ypp_b{b}t{tt}f{half}")
                    for cc in range(CCH - 1):
                        nc.tensor.matmul(
                            ps[:, 0:PH],
                            lhsT=mm(aoT_all[b][cc][:, tt * 128:(tt + 1) * 128]),
                            rhs=mm(wproj_sb[cc][:, half * PH:(half + 1) * PH]),
                            start=(cc == 0), stop=(cc == CCH - 2))
                    pp = yp.tile([128, PH], SD, tag=f"pp{tt}_{half}", bufs=1,
                                 name=f"pp_b{b}t{tt}f{half}")
                    with tc.high_priority(offset=300):
                        nc.vector.tensor_add(pp, ps[:, 0:PH],
                                             bias_sb[:, half * PH:(half + 1) * PH])
                    partials.append(pp)
            cl = CCH - 1
            for tt in range(NTT):
                for half in range(2):
                    ps = ps_w.tile([128, 512], FP32, tag="w",
                                   name=f"yps_b{b}t{tt}f{half}")
                    nc.tensor.matmul(
                        ps[:, 0:PH],
                        lhsT=mm(aoT_all[b][cl][:, tt * 128:(tt + 1) * 128]),
                        rhs=mm(wproj_sb[cl][:, half * PH:(half + 1) * PH]),
                        start=True, stop=True)
                    yt = yp.tile([128, PH], FP32, tag="y", name=f"y_b{b}t{tt}f{half}")
                    with tc.high_priority(offset=300):
                        nc.vector.tensor_add(yt, ps[:, 0:PH],
                                             partials[tt * 2 + half])
                    nc.sync.dma_start(
                        out=out_d[b * n + tt * 128:b * n + (tt + 1) * 128,
                                  half * PH:(half + 1) * PH],
                        in_=yt)

    nc.compile()
    return nc


_NC_CACHE = {}


def _get_nc(compute=COMPUTE):
    if compute not in _NC_CACHE:
        _NC_CACHE[compute] = build_attention_nc(compute)
    return _NC_CACHE[compute]


def make_in_maps(x, W_qkv, W_proj, b_proj, compute=None):
    compute = compute or COMPUTE
    if compute == "bf16":
        import ml_dtypes
        sd = ml_dtypes.bfloat16
    else:
        sd = np.float32
    x = np.asarray(x, dtype=np.float32)
    W_qkv = np.ascontiguousarray(np.asarray(W_qkv, dtype=np.float32)).astype(sd)
    W_proj = np.ascontiguousarray(np.asarray(W_proj, dtype=np.float32)).astype(sd)
    bias = np.ascontiguousarray(
        np.broadcast_to(np.asarray(b_proj, dtype=np.float32), (128, C)))
    in_maps = []
    for i in range(NCORES):
        shard = x[i * BL:(i + 1) * BL]                      # [BL, N, C]
        xT = np.ascontiguousarray(shard.transpose(2, 0, 1).reshape(C, T)).astype(sd)
        in_maps.append({"xT": xT, "w_qkv": W_qkv, "w_proj": W_proj,
                        "bias": bias})
    return in_maps


def kernel(x, W_qkv, W_proj, b_proj):
    from concourse.bass_utils import run_bass_kernel_spmd

    nc = _get_nc()
    in_maps = make_in_maps(x, W_qkv, W_proj, b_proj)
    res = run_bass_kernel_spmd(nc, in_maps, core_ids=list(range(NCORES)))
    outs = [res.results[i]["out"].reshape(BL, N, C) for i in range(NCORES)]
    return np.concatenate(outs, axis=0).astype(np.float32)


if __name__ == "__main__":
    nc = build_attention_nc()
    print("built ok")
